# revision 9
# baseline (speedup 1.0000x reference)
"""Multi-head attention (B=2, S=2048, D=1024, H=16) on 8 TRN2 NeuronCores.

Sharding: core c handles batch b = c//4 and head-group g = c%4 (4 heads,
d-slice of 256). Per core, on device:
  Q^T = WqT.T @ X^T (+bq)          [256, 2048]   (fp32 matmul, exact)
  K^T = WkT.T @ Xkv^T (+bk)        [256, SKV]
  V   = Xkv^T.T-chunks @ WvT       [SKV, 256]    ([k, d] layout)
  per head h, q-block: S^T = K_h Q_h^T, P = exp(S^T/8)  (f32r matmuls)
  psumO = [V_h | valid].T-style matmul accumulating [65, 512]:
     rows 0..63 = unnormalized O^T, row 64 = softmax denominator
  O^T = psumO[0:64] * (1/denominator)  (recip on DVE, partition-broadcast
     via GpSimd)
  OUT_partial = O^T.T @ WoT        [2048, 1024]  (fp32, DMA from PSUM)

Host side: keys/values are compacted by the attention mask (exact: masked
keys contribute exp->0 in the fp32 reference), padded to a multiple of 128;
the valid-flag column excludes padding from numerator and denominator.
V/O biases fold into a constant added on the host: A@(V+bv)Wo^T + bo =
A@V@Wo^T + (bv@Wo^T + bo). Partial outputs over head-groups are summed on
the host.
"""

import math
import os
from functools import lru_cache

import numpy as np

D_MODEL = 1024
NUM_HEADS = 16
D_K = 64
B = 2
S = 2048
N_CORES = 8
GROUPS = 4          # head-groups = cores per batch
DH = 256            # d-slice per core (4 heads x 64)
NH_LOC = 4          # heads per core
P = 128
CC = D_MODEL // P   # contraction chunks

# results of the last hardware run (BassKernelResults), for test harnesses
last_results = None


@lru_cache(maxsize=2)
def _build(SKV: int):
    import concourse.mybir as mybir
    import concourse.tile as tile
    from concourse import bacc

    f32 = mybir.dt.float32
    f32r = mybir.dt.float32r
    KC = SKV // P
    QB = S // 512                       # q blocks of 512
    kc_groups = [list(range(g, min(g + 3, KC))) for g in range(0, KC, 3)]
    kbs = [(s0, min(512, SKV - s0)) for s0 in range(0, SKV, 512)]

    nc = bacc.Bacc("TRN2", target_bir_lowering=False, debug=False,
                   num_devices=N_CORES)

    XT_d = nc.dram_tensor("xt", [D_MODEL, S], f32, kind="ExternalInput").ap()
    XKV_d = nc.dram_tensor("xkv", [D_MODEL, SKV], f32, kind="ExternalInput").ap()
    WQT_d = nc.dram_tensor("wqt", [D_MODEL, DH], f32, kind="ExternalInput").ap()
    WKT_d = nc.dram_tensor("wkt", [D_MODEL, DH], f32, kind="ExternalInput").ap()
    WVT_d = nc.dram_tensor("wvt", [D_MODEL, DH], f32, kind="ExternalInput").ap()
    WOT_d = nc.dram_tensor("wot", [DH, D_MODEL], f32, kind="ExternalInput").ap()
    bq_d = nc.dram_tensor("bq", [DH], f32, kind="ExternalInput").ap()
    bk_d = nc.dram_tensor("bk", [DH], f32, kind="ExternalInput").ap()
    vf_d = nc.dram_tensor("vf", [SKV], f32r, kind="ExternalInput").ap()
    OUT_d = nc.dram_tensor("out", [S, D_MODEL], f32, kind="ExternalOutput").ap()
    debug = bool(os.environ.get("KERNEL_DEBUG"))
    if debug:
        dQT = nc.dram_tensor("dbg_qt", [P, 2, S], f32r, kind="ExternalOutput").ap()
        dKT = nc.dram_tensor("dbg_kt", [P, 2, SKV], f32r, kind="ExternalOutput").ap()
        dV = nc.dram_tensor("dbg_v", [P, KC, NH_LOC, 65], f32r, kind="ExternalOutput").ap()
        dOT = nc.dram_tensor("dbg_ot", [P, 2, S], f32, kind="ExternalOutput").ap()
        dPSS = nc.dram_tensor("dbg_pss", [P, 3, 512], f32, kind="ExternalOutput").ap()
        dPTT = nc.dram_tensor("dbg_ptt", [P, 3, 512], f32r, kind="ExternalOutput").ap()
        dPSO = nc.dram_tensor("dbg_pso", [65, 512], f32, kind="ExternalOutput").ap()
        dREC = nc.dram_tensor("dbg_rec", [1, 512], f32, kind="ExternalOutput").ap()
        dRECB = nc.dram_tensor("dbg_recb", [64, 512], f32, kind="ExternalOutput").ap()

    with tile.TileContext(nc) as tc:
        with tc.tile_pool(name="res", bufs=1) as res:
            XKV_sb = res.tile([P, CC, SKV], f32)
            WQT_sb = res.tile([P, CC, DH], f32)
            WKT_sb = res.tile([P, CC, DH], f32)
            WVT_sb = res.tile([P, CC, DH], f32)
            WOT_sb = res.tile([P, 2, D_MODEL], f32)
            bq_sb = res.tile([P, 2], f32)
            bk_sb = res.tile([P, 2], f32)
            QT_sb = res.tile([P, 2, S], f32r)
            KT_sb = res.tile([P, 2, SKV], f32r)
            V_sb = res.tile([P, KC, NH_LOC, 65], f32r)
            OT_sb = res.tile([P, 2, S], f32)

            nc.sync.dma_start(XKV_sb[:], XKV_d.rearrange("(c p) k -> p c k", p=P))
            nc.sync.dma_start(WQT_sb[:], WQT_d.rearrange("(c p) d -> p c d", p=P))
            nc.sync.dma_start(WKT_sb[:], WKT_d.rearrange("(c p) d -> p c d", p=P))
            nc.sync.dma_start(WVT_sb[:], WVT_d.rearrange("(c p) d -> p c d", p=P))
            nc.sync.dma_start(WOT_sb[:], WOT_d.rearrange("(t p) e -> p t e", p=P))
            nc.sync.dma_start(bq_sb[:], bq_d.rearrange("(t p) -> p t", p=P))
            nc.sync.dma_start(bk_sb[:], bk_d.rearrange("(t p) -> p t", p=P))
            for h in range(NH_LOC):
                nc.sync.dma_start(V_sb[:, :, h, 64],
                                  vf_d.rearrange("(kc p) -> p kc", p=P))

            # ---------------- phase 1: projections (fp32) ----------------
            with tc.tile_pool(name="xs", bufs=2) as xs, \
                 tc.tile_pool(name="p1ps", bufs=4, space="PSUM") as p1ps:
                # V in [k, d'] layout: lhsT = Xkv^T chunk, rhs = WvT
                for kc in range(KC):
                    psv = p1ps.tile([P, 512], mybir.dt.float32, tag="p1")
                    for cc in range(CC):
                        nc.tensor.matmul(
                            psv[:, :DH],
                            XKV_sb[:, cc, kc * P:(kc + 1) * P],
                            WVT_sb[:, cc, :],
                            start=(cc == 0), stop=(cc == CC - 1))
                    nc.vector.tensor_copy(
                        V_sb[:, kc, :, 0:64],
                        psv[:, :DH].rearrange("p (h d) -> p h d", h=NH_LOC))
                # K^T: lhsT = WkT chunk, rhs = Xkv^T
                for (k0, sz) in kbs:
                    for t in range(2):
                        psk = p1ps.tile([P, 512], mybir.dt.float32, tag="p1")
                        for cc in range(CC):
                            nc.tensor.matmul(
                                psk[:, :sz],
                                WKT_sb[:, cc, t * P:(t + 1) * P],
                                XKV_sb[:, cc, k0:k0 + sz],
                                start=(cc == 0), stop=(cc == CC - 1))
                        nc.vector.tensor_scalar_add(
                            KT_sb[:, t, k0:k0 + sz], psk[:, :sz],
                            bk_sb[:, t:t + 1])
                # Q^T: stream X^T by q-block
                for qb in range(QB):
                    xt = xs.tile([P, CC, 512], f32, tag="xt")
                    nc.sync.dma_start(
                        xt[:],
                        XT_d.rearrange("(c p) q -> p c q", p=P)
                            [:, :, qb * 512:(qb + 1) * 512])
                    for t in range(2):
                        psq = p1ps.tile([P, 512], mybir.dt.float32, tag="p1")
                        for cc in range(CC):
                            nc.tensor.matmul(
                                psq[:],
                                WQT_sb[:, cc, t * P:(t + 1) * P],
                                xt[:, cc, :],
                                start=(cc == 0), stop=(cc == CC - 1))
                        nc.vector.tensor_scalar_add(
                            QT_sb[:, t, qb * 512:(qb + 1) * 512], psq[:],
                            bq_sb[:, t:t + 1])

            # ---------------- phase 2: attention (f32r) ----------------
            with tc.tile_pool(name="psS", bufs=2, space="PSUM") as psS, \
                 tc.tile_pool(name="psO", bufs=2, space="PSUM") as psO, \
                 tc.tile_pool(name="ptp", bufs=3) as ptp, \
                 tc.tile_pool(name="nrm", bufs=3) as nrm:
                for h in range(NH_LOC):
                    t, po = h // 2, (h % 2) * 64
                    for qb in range(QB):
                        q0 = qb * 512
                        pso = psO.tile([65, 512], mybir.dt.float32, tag="psO")
                        pts = []
                        for kcs in kc_groups:
                            pss = psS.tile([P, 3, 512], mybir.dt.float32,
                                           tag="psS")
                            for i, kc in enumerate(kcs):
                                nc.tensor.matmul(
                                    pss[:, i, :],
                                    KT_sb[po:po + 64, t, kc * P:(kc + 1) * P],
                                    QT_sb[po:po + 64, t, q0:q0 + 512],
                                    start=True, stop=True)
                            ptt = ptp.tile([P, 3, 512], f32r, tag="pt")
                            nc.scalar.activation(
                                ptt[:, :len(kcs), :], pss[:, :len(kcs), :],
                                mybir.ActivationFunctionType.Exp, scale=0.125)
                            pts.append((kcs, ptt))
                            if debug and h == 0 and qb == 0 and kcs[0] == 0:
                                dbs = nrm.tile([P, 3, 512], f32, tag="dbs")
                                nc.vector.tensor_copy(dbs[:], pss[:])
                                nc.sync.dma_start(dPSS, dbs[:])
                                nc.sync.dma_start(dPTT, ptt[:])
                        n_av = KC
                        j = 0
                        for kcs, ptt in pts:
                            for i, kc in enumerate(kcs):
                                nc.tensor.matmul(
                                    pso[:],
                                    V_sb[:, kc, h, :],
                                    ptt[:, i, :],
                                    start=(j == 0), stop=(j == n_av - 1))
                                j += 1
                        den = nrm.tile([1, 512], mybir.dt.float32, tag="den")
                        nc.vector.tensor_copy(den[:], pso[64:65, :])
                        rec = nrm.tile([1, 512], mybir.dt.float32, tag="rec")
                        nc.vector.reciprocal_approx_fast(rec[:], den[:])
                        recb = nrm.tile([64, 512], mybir.dt.float32, tag="recb")
                        nc.gpsimd.partition_broadcast(recb[:], rec[:],
                                                      channels=64)
                        if debug and h == 0 and qb == 0:
                            dbo = nrm.tile([65, 512], f32, tag="dbo")
                            nc.vector.tensor_copy(dbo[:], pso[:])
                            nc.sync.dma_start(dPSO, dbo[:])
                            nc.sync.dma_start(dREC, rec[:])
                            nc.sync.dma_start(dRECB, recb[:])
                        nc.vector.tensor_mul(
                            OT_sb[po:po + 64, t, q0:q0 + 512],
                            pso[0:64, :], recb[:])

            if debug:
                nc.sync.dma_start(dQT, QT_sb[:])
                nc.sync.dma_start(dKT, KT_sb[:])
                nc.sync.dma_start(dV, V_sb[:])
                nc.sync.dma_start(dOT, OT_sb[:])

            # ---------------- phase 3: output projection (fp32) ----------
            with tc.tile_pool(name="p3ps", bufs=2, space="PSUM") as p3ps, \
                 tc.tile_pool(name="p3sb", bufs=3) as p3sb:
                for qc in range(S // P):
                    ps3 = p3ps.tile([P, 2, 512], mybir.dt.float32, tag="p3")
                    for nb in range(2):
                        for t in range(2):
                            nc.tensor.matmul(
                                ps3[:, nb, :],
                                OT_sb[:, t, qc * P:(qc + 1) * P],
                                WOT_sb[:, t, nb * 512:(nb + 1) * 512],
                                start=(t == 0), stop=(t == 1))
                    ob = p3sb.tile([P, 2, 512], mybir.dt.float32, tag="ob")
                    nc.vector.tensor_copy(ob[:], ps3[:])
                    nc.sync.dma_start(OUT_d[qc * P:(qc + 1) * P, :], ob[:])

    nc.compile()
    return nc


def kernel(X, mask, W_Q, b_Q, W_K, b_K, W_V, b_V, W_O, b_O):
    global last_results
    from concourse.bass_utils import run_bass_kernel_spmd

    X = np.ascontiguousarray(X, dtype=np.float32)
    mask2 = np.asarray(mask).reshape(B, S) != 0
    counts = mask2.sum(axis=1)
    assert counts.min() >= 1
    SKV = max(P, int(math.ceil(counts.max() / P)) * P)

    XT = np.ascontiguousarray(X.transpose(0, 2, 1))          # (B, D, S)
    XKV = np.zeros((B, D_MODEL, SKV), dtype=np.float32)
    VF = np.zeros((B, SKV), dtype=np.float32)
    for b in range(B):
        idx = np.nonzero(mask2[b])[0]
        XKV[b, :, :len(idx)] = XT[b][:, idx]
        VF[b, :len(idx)] = 1.0

    nc = _build(SKV)

    in_maps = []
    for c in range(N_CORES):
        b, g = divmod(c, GROUPS)
        sl = slice(g * DH, (g + 1) * DH)
        in_maps.append({
            "xt": XT[b],
            "xkv": XKV[b],
            "wqt": np.ascontiguousarray(W_Q[sl, :].T),
            "wkt": np.ascontiguousarray(W_K[sl, :].T),
            "wvt": np.ascontiguousarray(W_V[sl, :].T),
            "wot": np.ascontiguousarray(W_O[:, sl].T),
            "bq": np.ascontiguousarray(b_Q[sl]),
            "bk": np.ascontiguousarray(b_K[sl]),
            "vf": VF[b],
        })

    trace_cores = None
    if os.environ.get("BASS_TRACE"):
        trace_cores = [int(x) for x in
                       os.environ.get("BASS_TRACE_CORES", "0").split(",")]
    res = run_bass_kernel_spmd(nc, in_maps, core_ids=list(range(N_CORES)),
                               trace_cores=trace_cores)
    last_results = res

    const = np.asarray(b_V, np.float64) @ np.asarray(W_O, np.float64).T \
        + np.asarray(b_O, np.float64)
    out = np.zeros((B, S, D_MODEL), dtype=np.float64)
    for c in range(N_CORES):
        b = c // GROUPS
        out[b] += res.results[c]["out"].astype(np.float64)
    out += const[None, None, :]
    return out.astype(np.float32)


# revision 10
# speedup vs baseline: 1.8322x; 1.8322x over previous
"""Multi-head attention (B=2, S=2048, D=1024, H=16) on 8 TRN2 NeuronCores.

Sharding: core c handles batch b = c//4 and head-group g = c%4 (4 heads,
d-slice of 256). Per core, on device:
  Q^T = WqT.T @ X^T (+bq)          [256, 2048]   (fp32 matmul, exact)
  K^T = WkT.T @ Xkv^T (+bk)        [256, SKV]
  V   = Xkv^T.T-chunks @ WvT       [SKV, 256]    ([k, d] layout)
  per head h, q-block: S^T = K_h Q_h^T, P = exp(S^T/8)  (f32r matmuls)
  psumO = [V_h | valid].T-style matmul accumulating [65, 512]:
     rows 0..63 = unnormalized O^T, row 64 = softmax denominator
  O^T = psumO[0:64] * (1/denominator)  (recip on DVE, partition-broadcast
     via GpSimd)
  OUT_partial = O^T.T @ WoT        [2048, 1024]  (fp32, DMA from PSUM)

Host side: keys/values are compacted by the attention mask (exact: masked
keys contribute exp->0 in the fp32 reference), padded to a multiple of 128;
the valid-flag column excludes padding from numerator and denominator.
V/O biases fold into a constant added on the host: A@(V+bv)Wo^T + bo =
A@V@Wo^T + (bv@Wo^T + bo). Partial outputs over head-groups are summed on
the host.
"""

import math
import os
from functools import lru_cache

import numpy as np

D_MODEL = 1024
NUM_HEADS = 16
D_K = 64
B = 2
S = 2048
N_CORES = 8
GROUPS = 4          # head-groups = cores per batch
DH = 256            # d-slice per core (4 heads x 64)
NH_LOC = 4          # heads per core
P = 128
CC = D_MODEL // P   # contraction chunks

# results of the last hardware run (BassKernelResults), for test harnesses
last_results = None


@lru_cache(maxsize=2)
def _build(SKV: int):
    import concourse.mybir as mybir
    import concourse.tile as tile
    from concourse import bacc

    f32 = mybir.dt.float32
    f32r = mybir.dt.float32r
    KC = SKV // P
    QB = S // 512                       # q blocks of 512
    kc_groups = [list(range(g, min(g + 3, KC))) for g in range(0, KC, 3)]
    kbs = [(s0, min(512, SKV - s0)) for s0 in range(0, SKV, 512)]

    nc = bacc.Bacc("TRN2", target_bir_lowering=False, debug=False,
                   num_devices=N_CORES)

    XT_d = nc.dram_tensor("xt", [D_MODEL, S], f32r, kind="ExternalInput").ap()
    XKV_d = nc.dram_tensor("xkv", [D_MODEL, SKV], f32r, kind="ExternalInput").ap()
    WQT_d = nc.dram_tensor("wqt", [D_MODEL, DH], f32r, kind="ExternalInput").ap()
    WKT_d = nc.dram_tensor("wkt", [D_MODEL, DH], f32r, kind="ExternalInput").ap()
    WVT_d = nc.dram_tensor("wvt", [D_MODEL, DH], f32r, kind="ExternalInput").ap()
    WOT_d = nc.dram_tensor("wot", [DH, D_MODEL], f32r, kind="ExternalInput").ap()
    bq_d = nc.dram_tensor("bq", [DH], f32, kind="ExternalInput").ap()
    bk_d = nc.dram_tensor("bk", [DH], f32, kind="ExternalInput").ap()
    vf_d = nc.dram_tensor("vf", [SKV], f32r, kind="ExternalInput").ap()
    OUT_d = nc.dram_tensor("out", [S, D_MODEL], f32, kind="ExternalOutput").ap()
    debug = bool(os.environ.get("KERNEL_DEBUG"))
    if debug:
        dQT = nc.dram_tensor("dbg_qt", [P, 2, S], f32r, kind="ExternalOutput").ap()
        dKT = nc.dram_tensor("dbg_kt", [P, 2, SKV], f32r, kind="ExternalOutput").ap()
        dV = nc.dram_tensor("dbg_v", [P, KC, NH_LOC, 65], f32r, kind="ExternalOutput").ap()
        dOT = nc.dram_tensor("dbg_ot", [P, 2, S], f32, kind="ExternalOutput").ap()
        dPSS = nc.dram_tensor("dbg_pss", [P, 3, 512], f32, kind="ExternalOutput").ap()
        dPTT = nc.dram_tensor("dbg_ptt", [P, 3, 512], f32r, kind="ExternalOutput").ap()
        dPSO = nc.dram_tensor("dbg_pso", [65, 512], f32, kind="ExternalOutput").ap()
        dREC = nc.dram_tensor("dbg_rec", [1, 512], f32, kind="ExternalOutput").ap()
        dRECB = nc.dram_tensor("dbg_recb", [64, 512], f32, kind="ExternalOutput").ap()

    with tile.TileContext(nc) as tc:
        with tc.tile_pool(name="res", bufs=1) as res:
            XKV_sb = res.tile([P, CC, SKV], f32r)
            WQT_sb = res.tile([P, CC, DH], f32r)
            WKT_sb = res.tile([P, CC, DH], f32r)
            WVT_sb = res.tile([P, CC, DH], f32r)
            WOT_sb = res.tile([P, 2, D_MODEL], f32r)
            bq_sb = res.tile([P, 2], f32)
            bk_sb = res.tile([P, 2], f32)
            QT_sb = res.tile([P, 2, S], f32r)
            KT_sb = res.tile([P, 2, SKV], f32r)
            V_sb = res.tile([P, KC, NH_LOC, 65], f32r)
            OT_sb = res.tile([P, 2, S], f32r)

            nc.sync.dma_start(XKV_sb[:], XKV_d.rearrange("(c p) k -> p c k", p=P))
            nc.sync.dma_start(WQT_sb[:], WQT_d.rearrange("(c p) d -> p c d", p=P))
            nc.sync.dma_start(WKT_sb[:], WKT_d.rearrange("(c p) d -> p c d", p=P))
            nc.sync.dma_start(WVT_sb[:], WVT_d.rearrange("(c p) d -> p c d", p=P))
            nc.sync.dma_start(WOT_sb[:], WOT_d.rearrange("(t p) e -> p t e", p=P))
            nc.sync.dma_start(bq_sb[:], bq_d.rearrange("(t p) -> p t", p=P))
            nc.sync.dma_start(bk_sb[:], bk_d.rearrange("(t p) -> p t", p=P))
            for h in range(NH_LOC):
                nc.sync.dma_start(V_sb[:, :, h, 64],
                                  vf_d.rearrange("(kc p) -> p kc", p=P))

            # ---------------- phase 1: projections (fp32) ----------------
            with tc.tile_pool(name="xs", bufs=2) as xs, \
                 tc.tile_pool(name="p1ps", bufs=4, space="PSUM") as p1ps:
                # V in [k, d'] layout: lhsT = Xkv^T chunk, rhs = WvT
                for kc in range(KC):
                    psv = p1ps.tile([P, 512], mybir.dt.float32, tag="p1")
                    for cc in range(CC):
                        nc.tensor.matmul(
                            psv[:, :DH],
                            XKV_sb[:, cc, kc * P:(kc + 1) * P],
                            WVT_sb[:, cc, :],
                            start=(cc == 0), stop=(cc == CC - 1))
                    nc.vector.tensor_copy(
                        V_sb[:, kc, :, 0:64],
                        psv[:, :DH].rearrange("p (h d) -> p h d", h=NH_LOC))
                # K^T: lhsT = WkT chunk, rhs = Xkv^T
                for (k0, sz) in kbs:
                    for t in range(2):
                        psk = p1ps.tile([P, 512], mybir.dt.float32, tag="p1")
                        for cc in range(CC):
                            nc.tensor.matmul(
                                psk[:, :sz],
                                WKT_sb[:, cc, t * P:(t + 1) * P],
                                XKV_sb[:, cc, k0:k0 + sz],
                                start=(cc == 0), stop=(cc == CC - 1))
                        nc.vector.tensor_scalar_add(
                            KT_sb[:, t, k0:k0 + sz], psk[:, :sz],
                            bk_sb[:, t:t + 1])
                # Q^T: stream X^T by q-block
                for qb in range(QB):
                    xt = xs.tile([P, CC, 512], f32r, tag="xt")
                    nc.sync.dma_start(
                        xt[:],
                        XT_d.rearrange("(c p) q -> p c q", p=P)
                            [:, :, qb * 512:(qb + 1) * 512])
                    for t in range(2):
                        psq = p1ps.tile([P, 512], mybir.dt.float32, tag="p1")
                        for cc in range(CC):
                            nc.tensor.matmul(
                                psq[:],
                                WQT_sb[:, cc, t * P:(t + 1) * P],
                                xt[:, cc, :],
                                start=(cc == 0), stop=(cc == CC - 1))
                        nc.vector.tensor_scalar_add(
                            QT_sb[:, t, qb * 512:(qb + 1) * 512], psq[:],
                            bq_sb[:, t:t + 1])

            # ---------------- phase 2: attention (f32r) ----------------
            with tc.tile_pool(name="psS", bufs=2, space="PSUM") as psS, \
                 tc.tile_pool(name="psO", bufs=2, space="PSUM") as psO, \
                 tc.tile_pool(name="ptp", bufs=3) as ptp, \
                 tc.tile_pool(name="nrm", bufs=3) as nrm:
                for h in range(NH_LOC):
                    t, po = h // 2, (h % 2) * 64
                    for qb in range(QB):
                        q0 = qb * 512
                        pso = psO.tile([65, 512], mybir.dt.float32, tag="psO")
                        pts = []
                        for kcs in kc_groups:
                            pss = psS.tile([P, 3, 512], mybir.dt.float32,
                                           tag="psS")
                            for i, kc in enumerate(kcs):
                                nc.tensor.matmul(
                                    pss[:, i, :],
                                    KT_sb[po:po + 64, t, kc * P:(kc + 1) * P],
                                    QT_sb[po:po + 64, t, q0:q0 + 512],
                                    start=True, stop=True)
                            ptt = ptp.tile([P, 3, 512], f32r, tag="pt")
                            nc.scalar.activation(
                                ptt[:, :len(kcs), :], pss[:, :len(kcs), :],
                                mybir.ActivationFunctionType.Exp, scale=0.125)
                            pts.append((kcs, ptt))
                            if debug and h == 0 and qb == 0 and kcs[0] == 0:
                                dbs = nrm.tile([P, 3, 512], f32, tag="dbs")
                                nc.vector.tensor_copy(dbs[:], pss[:])
                                nc.sync.dma_start(dPSS, dbs[:])
                                nc.sync.dma_start(dPTT, ptt[:])
                        n_av = KC
                        j = 0
                        for kcs, ptt in pts:
                            for i, kc in enumerate(kcs):
                                nc.tensor.matmul(
                                    pso[:],
                                    V_sb[:, kc, h, :],
                                    ptt[:, i, :],
                                    start=(j == 0), stop=(j == n_av - 1))
                                j += 1
                        den = nrm.tile([1, 512], mybir.dt.float32, tag="den")
                        nc.vector.tensor_copy(den[:], pso[64:65, :])
                        rec = nrm.tile([1, 512], mybir.dt.float32, tag="rec")
                        nc.vector.reciprocal_approx_fast(rec[:], den[:])
                        recb = nrm.tile([64, 512], mybir.dt.float32, tag="recb")
                        nc.gpsimd.partition_broadcast(recb[:], rec[:],
                                                      channels=64)
                        if debug and h == 0 and qb == 0:
                            dbo = nrm.tile([65, 512], f32, tag="dbo")
                            nc.vector.tensor_copy(dbo[:], pso[:])
                            nc.sync.dma_start(dPSO, dbo[:])
                            nc.sync.dma_start(dREC, rec[:])
                            nc.sync.dma_start(dRECB, recb[:])
                        nc.vector.tensor_mul(
                            OT_sb[po:po + 64, t, q0:q0 + 512],
                            pso[0:64, :], recb[:])

            if debug:
                nc.sync.dma_start(dQT, QT_sb[:])
                nc.sync.dma_start(dKT, KT_sb[:])
                nc.sync.dma_start(dV, V_sb[:])
                nc.sync.dma_start(dOT, OT_sb[:])

            # ---------------- phase 3: output projection (fp32) ----------
            with tc.tile_pool(name="p3ps", bufs=2, space="PSUM") as p3ps, \
                 tc.tile_pool(name="p3sb", bufs=3) as p3sb:
                for qc in range(S // P):
                    ps3 = p3ps.tile([P, 2, 512], mybir.dt.float32, tag="p3")
                    for nb in range(2):
                        for t in range(2):
                            nc.tensor.matmul(
                                ps3[:, nb, :],
                                OT_sb[:, t, qc * P:(qc + 1) * P],
                                WOT_sb[:, t, nb * 512:(nb + 1) * 512],
                                start=(t == 0), stop=(t == 1))
                    ob = p3sb.tile([P, 2, 512], mybir.dt.float32, tag="ob")
                    nc.vector.tensor_copy(ob[:], ps3[:])
                    nc.sync.dma_start(OUT_d[qc * P:(qc + 1) * P, :], ob[:])

    nc.compile()
    return nc


def kernel(X, mask, W_Q, b_Q, W_K, b_K, W_V, b_V, W_O, b_O):
    global last_results
    from concourse.bass_utils import run_bass_kernel_spmd

    X = np.ascontiguousarray(X, dtype=np.float32)
    mask2 = np.asarray(mask).reshape(B, S) != 0
    counts = mask2.sum(axis=1)
    assert counts.min() >= 1
    SKV = max(P, int(math.ceil(counts.max() / P)) * P)

    XT = np.ascontiguousarray(X.transpose(0, 2, 1))          # (B, D, S)
    XKV = np.zeros((B, D_MODEL, SKV), dtype=np.float32)
    VF = np.zeros((B, SKV), dtype=np.float32)
    for b in range(B):
        idx = np.nonzero(mask2[b])[0]
        XKV[b, :, :len(idx)] = XT[b][:, idx]
        VF[b, :len(idx)] = 1.0

    nc = _build(SKV)

    in_maps = []
    for c in range(N_CORES):
        b, g = divmod(c, GROUPS)
        sl = slice(g * DH, (g + 1) * DH)
        in_maps.append({
            "xt": XT[b],
            "xkv": XKV[b],
            "wqt": np.ascontiguousarray(W_Q[sl, :].T),
            "wkt": np.ascontiguousarray(W_K[sl, :].T),
            "wvt": np.ascontiguousarray(W_V[sl, :].T),
            "wot": np.ascontiguousarray(W_O[:, sl].T),
            "bq": np.ascontiguousarray(b_Q[sl]),
            "bk": np.ascontiguousarray(b_K[sl]),
            "vf": VF[b],
        })

    trace_cores = None
    if os.environ.get("BASS_TRACE"):
        trace_cores = [int(x) for x in
                       os.environ.get("BASS_TRACE_CORES", "0").split(",")]
    res = run_bass_kernel_spmd(nc, in_maps, core_ids=list(range(N_CORES)),
                               trace_cores=trace_cores)
    last_results = res

    const = np.asarray(b_V, np.float64) @ np.asarray(W_O, np.float64).T \
        + np.asarray(b_O, np.float64)
    out = np.zeros((B, S, D_MODEL), dtype=np.float64)
    for c in range(N_CORES):
        b = c // GROUPS
        out[b] += res.results[c]["out"].astype(np.float64)
    out += const[None, None, :]
    return out.astype(np.float32)
